# revision 22
# baseline (speedup 1.0000x reference)
"""CBL (contrastive boundary) loss kernel for Trainium2, 8 NeuronCores.

Strategy (data-parallel over points, per spec sharding hint):
  - Shard the N=100000 points across 8 cores (12500 each, zero-padded to
    12544 = 128 partitions x 98 tiles).
  - The original kernel issued one SWDGE indirect DMA per (tile, k) pair:
    686 instructions x (994 ns fixed + 128*0.34 ns) ~= 711 us of pure
    Pool-engine descriptor-generation overhead -- the measured bottleneck.
  - This version uses batched `dma_gather` (InstDMAGatherAnt). Its Q7 ucode
    stages indices in a 1024-entry data scratch, so one instruction moves at
    most 1024 rows; each core issues 86 sub-gathers (vs 686 indirect DMAs).
  - Sub-gathers round-robin over 4 SWDGE queues: each queue has its own
    descriptor ring (and Q7 cpu pair), so desc-gen of gather n does not
    block on the ring space freed only when gather n-1's DMA completes
    (measured: single-queue ping-pong cost ~6.9 us per gather).
  - dma_gather indexes are int16 (< 32768), so each 16-tile chunk gathers
    from its own host-compacted table: the distinct neighbor rows referenced
    by that chunk, remapped to local int16 ids ("neighbor_idx values stay
    local per-shard", as the sharding hint suggests). Compaction is pure
    index marshaling (np.unique/searchsorted); all float math stays on-chip.
  - Neighbor labels (target[neighbor_idx]) ship as a marshaled input the
    same way the point labels do; features are gathered on-chip.
  - Tables are bf16 (256B rows): halves gather bytes and enables the DVE
    2x_1p mode (all-2-byte packed operands) for the big elementwise ops.
    DVE tensor_reduce has no fast mode, so the C=128 reductions run as
    bf16 halving-add trees (2x rate), with the final 2->1 step emitting f32.
  - cos = dot * rsqrt(ss_i) * rsqrt(ss_j); dist = sqrt(max(2-2cos,0)+eps)
    -- identical to the reference's normalized L2 distance.
  - NCE contrast / masking / per-pair loss evaluated on-chip; each core
    emits partial (sum, count); host combines the 8 pairs (the scalar
    "all-reduce" of the sharding hint) and forms sum/max(cnt,1).
"""

import sys

if "/opt/trn_rl_repo" not in sys.path:
    sys.path.insert(0, "/opt/trn_rl_repo")

import numpy as np

N_TOTAL = 100000
C = 128
K = 7
NCORES = 8
P = 128
NSH = N_TOTAL // NCORES          # 12500 points per core
EPS = 1e-12
FP16 = False                     # kept for test.py compatibility

NQUEUES = 4                      # SWDGE queues used for gathers
GMAX = 1024                      # max idxs per dma_gather (Q7 scratch limit)
T_TOT = 98                       # tiles per core (12544 points)
NPAD = T_TOT * P                 # 12544
TK = T_TOT * K                   # 686
# chunks of 16 tiles (112 slots = 14 exact 1024-idx sub-gathers) + one
# 2-tile tail chunk (14 slots = 1024 + 768)
CHUNK_TILES = [16, 16, 16, 16, 16, 16, 2]
NCHUNK = len(CHUNK_TILES)
CHUNK_T0 = np.cumsum([0] + CHUNK_TILES).tolist()


def build_nc(fp16=False):
    from concourse import bacc, bass  # noqa: F401
    import concourse.mybir as mybir
    from concourse.tile import TileContext

    f32 = mybir.dt.float32
    bf16 = mybir.dt.bfloat16
    i16 = mybir.dt.int16
    Alu = mybir.AluOpType
    Act = mybir.ActivationFunctionType
    Ax = mybir.AxisListType

    GGRP = max(CHUNK_TILES)      # 16 tiles per full chunk
    USZ = GGRP * K * P           # table rows per chunk (padded upper bound)

    nc = bacc.Bacc(num_devices=NCORES, num_swdge_queues=NQUEUES)
    xs = nc.dram_tensor("xs", [NPAD, C], bf16, kind="ExternalInput")
    tab = nc.dram_tensor("tab", [NCHUNK, USZ, C], bf16, kind="ExternalInput")
    idx16 = nc.dram_tensor("idx16", [NCHUNK, P, USZ // 16], i16,
                           kind="ExternalInput")
    tgn = nc.dram_tensor("tgn", [P, TK], f32, kind="ExternalInput")
    tgts = nc.dram_tensor("tgts", [P, T_TOT], f32, kind="ExternalInput")
    part = nc.dram_tensor("part", [2], f32, kind="ExternalOutput")

    qcnt = [0]

    def halving_tree(eng, buf, out_f32, reshape=lambda ap: ap):
        """Sum over the last (C) axis of buf [P, S, C] (bf16) via in-place
        halving adds (DVE 2x_1p); final 2->1 step emits f32 into out_f32,
        with `reshape` applied to the final bf16 input slices to match."""
        w = C // 2
        while w >= 1:
            if w == 1:
                eng.tensor_tensor(out=out_f32, in0=reshape(buf[:, :, 0:1]),
                                  in1=reshape(buf[:, :, 1:2]), op=Alu.add)
            else:
                eng.tensor_tensor(out=buf[:, :, 0:w], in0=buf[:, :, 0:w],
                                  in1=buf[:, :, w:2 * w], op=Alu.add)
            w //= 2

    with TileContext(nc) as tc:
        with (
            tc.tile_pool(name="cst", bufs=1) as cst,
            tc.tile_pool(name="nbrp", bufs=3) as nbrp,
            tc.tile_pool(name="xsp", bufs=2) as xsp,
            tc.tile_pool(name="idxp", bufs=2) as idxp,
            tc.tile_pool(name="sqnp", bufs=2) as sqnp,
            tc.tile_pool(name="prodp", bufs=1) as prodp,
            tc.tile_pool(name="sqxp", bufs=2) as sqxp,
            tc.tile_pool(name="psp", bufs=1, space="PSUM") as psp,
        ):
            # ---- resident loads ----
            tgn_sb = cst.tile([P, TK], f32)
            tgts_sb = cst.tile([P, T_TOT], f32)
            nc.sync.dma_start(out=tgn_sb[:], in_=tgn[:, :])
            nc.sync.dma_start(out=tgts_sb[:], in_=tgts[:, :])

            dot_all = cst.tile([P, TK], f32)
            ssn_all = cst.tile([P, TK], f32)
            ssi_all = cst.tile([P, T_TOT], f32)

            # ---- gather + per-pair dot/norm, chunk-pipelined ----
            for g in range(NCHUNK):
                grp = CHUNK_TILES[g]
                t0 = CHUNK_T0[g]
                slots = grp * K
                nidx = slots * P

                xs_t = xsp.tile([P, GGRP, C], bf16, tag="xs")
                nc.sync.dma_start(
                    out=xs_t[:, 0:grp, :],
                    in_=xs[t0 * P:(t0 + grp) * P, :]
                    .rearrange("(p t) c -> p t c", t=grp))
                idx_t = idxp.tile([P, USZ // 16], i16, tag="idx")
                nc.sync.dma_start(out=idx_t[:, 0:nidx // 16],
                                  in_=idx16[g, :, 0:nidx // 16])

                nbr = nbrp.tile([P, GGRP * K, C], bf16, tag="nbr")
                for s in range(0, nidx, GMAX):
                    n = min(GMAX, nidx - s)
                    nc.gpsimd.dma_gather(
                        out_ap=nbr[:, s // P:(s + n) // P, :],
                        in_ap=tab[g, :, :],
                        idxs_ap=idx_t[:, s // 16:(s + n) // 16],
                        num_idxs=n,
                        num_idxs_reg=n,
                        elem_size=C,
                        queue_num=qcnt[0] % NQUEUES,
                    )
                    qcnt[0] += 1

                # self sum-of-squares for this chunk's points
                sqx = sqxp.tile([P, GGRP, C], bf16, tag="sqx")
                nc.scalar.activation(out=sqx[:, 0:grp, :], in_=xs_t[:, 0:grp, :],
                                     func=Act.Square)
                nc.vector.tensor_reduce(
                    out=ssi_all[:, t0:t0 + grp],
                    in_=sqx[:, 0:grp, :], axis=Ax.X, op=Alu.add)

                # neighbor sum-of-squares: ACT squares, Pool halving tree
                sqn = sqnp.tile([P, GGRP * K, C], bf16, tag="sqn")
                nc.scalar.activation(out=sqn[:, 0:slots, :],
                                     in_=nbr[:, 0:slots, :], func=Act.Square)
                halving_tree(nc.gpsimd, sqn[:, 0:slots, :],
                             ssn_all[:, t0 * K:(t0 + grp) * K]
                             .rearrange("p (s o) -> p s o", o=1))

                # dot(x_i, x_j): DVE bf16 product + DVE halving tree
                prod = prodp.tile([P, GGRP * K, C], bf16, tag="prod")
                nc.vector.tensor_tensor(
                    out=prod[:, 0:slots, :].rearrange("p (t k) c -> p t k c", k=K),
                    in0=nbr[:, 0:slots, :].rearrange("p (t k) c -> p t k c", k=K),
                    in1=xs_t[:, 0:grp, None, :].to_broadcast([P, grp, K, C]),
                    op=Alu.mult)
                halving_tree(nc.vector, prod[:, 0:slots, :],
                             dot_all[:, t0 * K:(t0 + grp) * K]
                             .rearrange("p (s o) -> p s o", o=1))

            # ---- phase B: per-pair loss on [P, TK] ----
            def seg(ap):
                return ap.rearrange("p (t k) -> p t k", k=K)

            # r_i = sqrt(1/(ss_i + eps))
            r_sb = cst.tile([P, T_TOT], f32)
            nc.vector.tensor_scalar_add(ssi_all[:], ssi_all[:], EPS)
            nc.vector.reciprocal(ssi_all[:], ssi_all[:])
            nc.scalar.activation(out=r_sb[:], in_=ssi_all[:], func=Act.Sqrt)

            # rn_j = sqrt(1/(ss_j + eps))
            rn = cst.tile([P, TK], f32)
            nc.vector.tensor_scalar_add(ssn_all[:], ssn_all[:], EPS)
            nc.vector.reciprocal(ssn_all[:], ssn_all[:])
            nc.scalar.activation(out=rn[:], in_=ssn_all[:], func=Act.Sqrt)

            # cos -> d2 -> dist (rn reused as scratch chain)
            nc.vector.tensor_tensor(out=rn[:], in0=dot_all[:], in1=rn[:],
                                    op=Alu.mult)
            nc.vector.tensor_tensor(out=seg(rn), in0=seg(rn),
                                    in1=r_sb[:, :, None].to_broadcast([P, T_TOT, K]),
                                    op=Alu.mult)
            nc.vector.tensor_scalar(rn[:], rn[:], -2.0, 2.0, Alu.mult, Alu.add)
            nc.vector.tensor_scalar_max(rn[:], rn[:], 0.0)
            eps_tile = cst.tile([P, 1], f32)
            nc.vector.memset(eps_tile[:], EPS)
            dist = dot_all  # dot no longer needed
            nc.scalar.activation(out=dist[:], in_=rn[:], func=Act.Sqrt,
                                 bias=eps_tile[:, 0:1])

            # M = -min_k dist; s = dist + M; e = exp(-s)
            M = cst.tile([P, T_TOT], f32)
            nc.vector.tensor_reduce(out=M[:], in_=seg(dist), axis=Ax.X,
                                    op=Alu.min, negate=True)
            s_t = dist
            nc.vector.tensor_tensor(out=seg(s_t), in0=seg(dist),
                                    in1=M[:, :, None].to_broadcast([P, T_TOT, K]),
                                    op=Alu.add)
            e_t = cst.tile([P, TK], f32)
            nc.scalar.activation(out=e_t[:], in_=s_t[:], func=Act.Exp, scale=-1.0)

            # posmask, npos, point_mask
            pos = tgn_sb  # overwrite labels with the mask
            nc.vector.tensor_tensor(out=seg(pos), in0=seg(tgn_sb),
                                    in1=tgts_sb[:, :, None].to_broadcast([P, T_TOT, K]),
                                    op=Alu.is_equal)
            npos = cst.tile([P, T_TOT], f32)
            nc.vector.tensor_reduce(out=npos[:], in_=seg(pos), axis=Ax.X, op=Alu.add)
            g1 = cst.tile([P, T_TOT], f32)
            pm = cst.tile([P, T_TOT], f32)
            nc.vector.tensor_scalar(g1[:], npos[:], 0.5, None, Alu.is_gt)
            nc.vector.tensor_scalar(pm[:], npos[:], K - 0.5, None, Alu.is_lt)
            nc.vector.tensor_tensor(out=pm[:], in0=g1[:], in1=pm[:], op=Alu.mult)

            # neg = sum(e) - sum(e*pos); under = e + neg; L = ln(under)
            sall = cst.tile([P, T_TOT], f32)
            nc.vector.tensor_reduce(out=sall[:], in_=seg(e_t), axis=Ax.X, op=Alu.add)
            ep = ssn_all  # scratch
            nc.vector.tensor_tensor(out=ep[:], in0=e_t[:], in1=pos[:], op=Alu.mult)
            spos = cst.tile([P, T_TOT], f32)
            nc.vector.tensor_reduce(out=spos[:], in_=seg(ep), axis=Ax.X, op=Alu.add)
            nc.vector.tensor_tensor(out=sall[:], in0=sall[:], in1=spos[:],
                                    op=Alu.subtract)
            nc.vector.tensor_tensor(out=seg(e_t), in0=seg(e_t),
                                    in1=sall[:, :, None].to_broadcast([P, T_TOT, K]),
                                    op=Alu.add)
            nc.scalar.activation(out=e_t[:], in_=e_t[:], func=Act.Ln)

            # per_pair = L + s ; contrib = per_pair * pos * pm
            nc.vector.tensor_tensor(out=e_t[:], in0=e_t[:], in1=s_t[:], op=Alu.add)
            nc.vector.tensor_tensor(out=seg(pos), in0=seg(pos),
                                    in1=pm[:, :, None].to_broadcast([P, T_TOT, K]),
                                    op=Alu.mult)
            nc.vector.tensor_tensor(out=e_t[:], in0=e_t[:], in1=pos[:], op=Alu.mult)

            # reduce to per-partition (sum, cnt), then across partitions via PE
            vals = cst.tile([P, 2], f32)
            nc.vector.tensor_reduce(out=vals[:, 0:1], in_=e_t[:], axis=Ax.X,
                                    op=Alu.add)
            nc.vector.tensor_reduce(out=vals[:, 1:2], in_=pos[:], axis=Ax.X,
                                    op=Alu.add)
            ones = cst.tile([P, 1], f32)
            nc.vector.memset(ones[:], 1.0)
            pst = psp.tile([2, 1], f32, space="PSUM")
            nc.tensor.matmul(out=pst[:], lhsT=vals[:], rhs=ones[:], start=True,
                             stop=True)
            res_sb = cst.tile([2, 1], f32)
            nc.vector.tensor_copy(out=res_sb[:], in_=pst[:])
            nc.sync.dma_start(out=part[:], in_=res_sb[:])
    nc.finalize()
    return nc


def make_in_maps(x, neighbor_idx, target, nsh=NSH, ncores=NCORES, fp16=False):
    """Shard + pad + per-chunk index compaction, host-side (data marshaling)."""
    import ml_dtypes

    bf = ml_dtypes.bfloat16
    x = np.ascontiguousarray(np.asarray(x, dtype=np.float32)).astype(bf)
    idx_all = np.asarray(neighbor_idx).astype(np.int64)
    tgtf = np.asarray(target).astype(np.float32)

    GGRP = max(CHUNK_TILES)
    USZ = GGRP * K * P

    in_maps = []
    for c in range(ncores):
        lo = c * nsh
        pts = np.arange(lo, lo + NPAD, dtype=np.int64)
        valid = pts < lo + nsh
        pts = np.where(valid, pts, lo)  # pad points alias point `lo`

        # point (p, t) of chunk g at xs row (t0 + t_l)*P ... laid out so the
        # chunk's block is contiguous: row = t0*P + p*grp + t_l
        xs_host = np.zeros((NPAD, C), dtype=bf)
        tgts_host = np.full((P, T_TOT), -1.0, dtype=np.float32)
        tgn_host = np.zeros((P, T_TOT, K), dtype=np.float32)
        tab_host = np.zeros((NCHUNK, USZ, C), dtype=bf)
        idx16_host = np.zeros((NCHUNK, P, USZ // 16), dtype=np.int16)

        for g, grp in enumerate(CHUNK_TILES):
            t0 = CHUNK_T0[g]
            nidx = grp * K * P
            # u in [0, grp*P): u -> (p = u//grp, t_l = u%grp)
            u = np.arange(grp * P)
            p_g, tl_g = u // grp, u % grp
            gpts = pts[t0 * P + u]
            xs_host[t0 * P + u] = x[gpts]
            t_glob = t0 + tl_g
            tgts_host[p_g, t_glob] = np.where(valid[t0 * P + u],
                                              tgtf[gpts], -1.0)
            refs = idx_all[gpts]                       # [grp*P, K]
            tgn_host[p_g, t_glob, :] = tgtf[refs]

            uniq = np.unique(refs)
            assert uniq.size <= USZ and uniq.size < 32768
            tab_host[g, :uniq.size] = x[uniq]
            lidx = np.searchsorted(uniq, refs).astype(np.int16)
            # gather position i = slot*128 + p, slot = t_l*K + k
            flat = np.zeros(nidx, dtype=np.int16)
            slot = tl_g[:, None] * K + np.arange(K)[None, :]
            flat[slot * P + p_g[:, None]] = lidx
            idx16_host[g, :, 0:nidx // 16] = np.tile(
                flat.reshape(nidx // 16, 16).T, (8, 1))

        in_maps.append({
            "xs": xs_host,
            "tab": tab_host,
            "idx16": idx16_host,
            "tgn": tgn_host.reshape(P, TK),
            "tgts": tgts_host,
        })
    return in_maps


def combine_parts(parts):
    parts = np.asarray(parts, dtype=np.float64)
    s = parts[:, 0].sum()
    cnt = parts[:, 1].sum()
    loss = s / max(cnt, 1.0) if cnt > 0 else 0.0
    return np.asarray(loss, dtype=np.float32)


def kernel(p, x, neighbor_idx, target):
    from concourse.bass_utils import run_bass_kernel_spmd

    in_maps = make_in_maps(x, neighbor_idx, target, fp16=FP16)
    nc = build_nc(fp16=FP16)
    res = run_bass_kernel_spmd(nc, in_maps, list(range(NCORES)))
    parts = [r["part"] for r in res.results]
    return combine_parts(parts)


# revision 23
# speedup vs baseline: 2.3475x; 2.3475x over previous
"""CBL (contrastive boundary) loss kernel for Trainium2, 8 NeuronCores.

Strategy (data-parallel over points, per spec sharding hint):
  - Shard the N=100000 points across 8 cores (12500 each, zero-padded to
    12544 = 128 partitions x 98 tiles).
  - The original kernel issued one SWDGE indirect DMA per (tile, k) pair:
    686 instructions x (994 ns fixed + 128*0.34 ns) ~= 711 us of pure
    Pool-engine descriptor-generation overhead -- the measured bottleneck.
  - This version uses batched `dma_gather` (InstDMAGatherAnt). Its Q7 ucode
    stages indices in a 1024-entry data scratch, so one instruction moves at
    most 1024 rows; each core issues 86 sub-gathers (vs 686 indirect DMAs).
  - Sub-gathers round-robin over 4 SWDGE queues: each queue has its own
    descriptor ring (and Q7 cpu pair), so desc-gen of gather n does not
    block on the ring space freed only when gather n-1's DMA completes
    (measured: single-queue ping-pong cost ~6.9 us per gather).
  - dma_gather indexes are int16 (< 32768), so each 16-tile chunk gathers
    from its own host-compacted table: the distinct neighbor rows referenced
    by that chunk, remapped to local int16 ids ("neighbor_idx values stay
    local per-shard", as the sharding hint suggests). Compaction is pure
    index marshaling (np.unique/searchsorted); all float math stays on-chip.
  - Neighbor labels (target[neighbor_idx]) ship as a marshaled input the
    same way the point labels do; features are gathered on-chip.
  - Tables are bf16 (256B rows): halves gather bytes and enables the DVE
    2x_1p mode (all-2-byte packed operands) for the big elementwise ops.
    DVE tensor_reduce has no fast mode, so the C=128 reductions run as
    bf16 halving-add trees (2x rate), with the final 2->1 step emitting f32.
  - cos = dot * rsqrt(ss_i) * rsqrt(ss_j); dist = sqrt(max(2-2cos,0)+eps)
    -- identical to the reference's normalized L2 distance.
  - NCE contrast / masking / per-pair loss evaluated on-chip; each core
    emits partial (sum, count); host combines the 8 pairs (the scalar
    "all-reduce" of the sharding hint) and forms sum/max(cnt,1).
"""

import sys

if "/opt/trn_rl_repo" not in sys.path:
    sys.path.insert(0, "/opt/trn_rl_repo")

import numpy as np

N_TOTAL = 100000
C = 128
K = 7
NCORES = 8
P = 128
NSH = N_TOTAL // NCORES          # 12500 points per core
EPS = 1e-12
FP16 = False                     # kept for test.py compatibility

NQUEUES = 4                      # SWDGE queues used for gathers
GMAX = 1024                      # max idxs per dma_gather (Q7 scratch limit)
T_TOT = 98                       # tiles per core (12544 points)
NPAD = T_TOT * P                 # 12544
TK = T_TOT * K                   # 686
# chunks of 16 tiles (112 slots = 14 exact 1024-idx sub-gathers) + one
# 2-tile tail chunk (14 slots = 1024 + 768)
CHUNK_TILES = [16, 16, 16, 16, 16, 16, 2]
NCHUNK = len(CHUNK_TILES)
CHUNK_T0 = np.cumsum([0] + CHUNK_TILES).tolist()


def build_nc(fp16=False):
    from concourse import bacc, bass  # noqa: F401
    import concourse.mybir as mybir
    from concourse.tile import TileContext

    f32 = mybir.dt.float32
    bf16 = mybir.dt.bfloat16
    i16 = mybir.dt.int16
    Alu = mybir.AluOpType
    Act = mybir.ActivationFunctionType
    Ax = mybir.AxisListType

    GGRP = max(CHUNK_TILES)      # 16 tiles per full chunk
    USZ = GGRP * K * P           # table rows per chunk (padded upper bound)

    nc = bacc.Bacc(num_devices=NCORES, num_swdge_queues=NQUEUES)
    xs = nc.dram_tensor("xs", [NPAD, C], bf16, kind="ExternalInput")
    tab = nc.dram_tensor("tab", [NCHUNK, USZ, C], bf16, kind="ExternalInput")
    idx16 = nc.dram_tensor("idx16", [NCHUNK, P, USZ // 16], i16,
                           kind="ExternalInput")
    tgn = nc.dram_tensor("tgn", [P, TK], f32, kind="ExternalInput")
    tgts = nc.dram_tensor("tgts", [P, T_TOT], f32, kind="ExternalInput")
    part = nc.dram_tensor("part", [2], f32, kind="ExternalOutput")

    qcnt = [0]

    def halving_tree(eng, buf, out_f32, reshape=lambda ap: ap):
        """Sum over the last (C) axis of buf [P, S, C] (bf16) via in-place
        halving adds (DVE 2x_1p); final 2->1 step emits f32 into out_f32,
        with `reshape` applied to the final bf16 input slices to match."""
        w = C // 2
        while w >= 1:
            if w == 1:
                eng.tensor_tensor(out=out_f32, in0=reshape(buf[:, :, 0:1]),
                                  in1=reshape(buf[:, :, 1:2]), op=Alu.add)
            else:
                eng.tensor_tensor(out=buf[:, :, 0:w], in0=buf[:, :, 0:w],
                                  in1=buf[:, :, w:2 * w], op=Alu.add)
            w //= 2

    with TileContext(nc) as tc:
        with (
            tc.tile_pool(name="cst", bufs=1) as cst,
            tc.tile_pool(name="nbrp", bufs=3) as nbrp,
            tc.tile_pool(name="xsp", bufs=2) as xsp,
            tc.tile_pool(name="idxp", bufs=2) as idxp,
            tc.tile_pool(name="sqnp", bufs=2) as sqnp,
            tc.tile_pool(name="prodp", bufs=1) as prodp,
            tc.tile_pool(name="sqxp", bufs=2) as sqxp,
            tc.tile_pool(name="psp", bufs=1, space="PSUM") as psp,
        ):
            # ---- resident loads ----
            tgn_sb = cst.tile([P, TK], f32)
            tgts_sb = cst.tile([P, T_TOT], f32)
            nc.sync.dma_start(out=tgn_sb[:], in_=tgn[:, :])
            nc.sync.dma_start(out=tgts_sb[:], in_=tgts[:, :])

            dot_all = cst.tile([P, TK], f32)
            ssn_all = cst.tile([P, TK], f32)
            ssi_all = cst.tile([P, T_TOT], f32)

            # ---- gather + per-pair dot/norm, chunk-pipelined ----
            for g in range(NCHUNK):
                grp = CHUNK_TILES[g]
                t0 = CHUNK_T0[g]
                slots = grp * K
                nidx = slots * P

                xs_t = xsp.tile([P, GGRP, C], bf16, tag="xs")
                nc.sync.dma_start(
                    out=xs_t[:, 0:grp, :],
                    in_=xs[t0 * P:(t0 + grp) * P, :]
                    .rearrange("(p t) c -> p t c", t=grp))
                idx_t = idxp.tile([P, USZ // 16], i16, tag="idx")
                nc.sync.dma_start(out=idx_t[:, 0:nidx // 16],
                                  in_=idx16[g, :, 0:nidx // 16])

                nbr = nbrp.tile([P, GGRP * K, C], bf16, tag="nbr")
                for s in range(0, nidx, GMAX):
                    n = min(GMAX, nidx - s)
                    nc.gpsimd.dma_gather(
                        out_ap=nbr[:, s // P:(s + n) // P, :],
                        in_ap=tab[g, :, :],
                        idxs_ap=idx_t[:, s // 16:(s + n) // 16],
                        num_idxs=n,
                        num_idxs_reg=n,
                        elem_size=C,
                        queue_num=qcnt[0] % NQUEUES,
                    )
                    qcnt[0] += 1

                # self sum-of-squares for this chunk's points
                sqx = sqxp.tile([P, GGRP, C], bf16, tag="sqx")
                nc.scalar.activation(out=sqx[:, 0:grp, :], in_=xs_t[:, 0:grp, :],
                                     func=Act.Square)
                nc.vector.tensor_reduce(
                    out=ssi_all[:, t0:t0 + grp],
                    in_=sqx[:, 0:grp, :], axis=Ax.X, op=Alu.add)

                # neighbor sum-of-squares: ACT squares, DVE halving tree
                sqn = sqnp.tile([P, GGRP * K, C], bf16, tag="sqn")
                nc.scalar.activation(out=sqn[:, 0:slots, :],
                                     in_=nbr[:, 0:slots, :], func=Act.Square)
                halving_tree(nc.vector, sqn[:, 0:slots, :],
                             ssn_all[:, t0 * K:(t0 + grp) * K]
                             .rearrange("p (s o) -> p s o", o=1))

                # dot(x_i, x_j): DVE bf16 product + DVE halving tree
                prod = prodp.tile([P, GGRP * K, C], bf16, tag="prod")
                nc.vector.tensor_tensor(
                    out=prod[:, 0:slots, :].rearrange("p (t k) c -> p t k c", k=K),
                    in0=nbr[:, 0:slots, :].rearrange("p (t k) c -> p t k c", k=K),
                    in1=xs_t[:, 0:grp, None, :].to_broadcast([P, grp, K, C]),
                    op=Alu.mult)
                halving_tree(nc.vector, prod[:, 0:slots, :],
                             dot_all[:, t0 * K:(t0 + grp) * K]
                             .rearrange("p (s o) -> p s o", o=1))

            # ---- phase B: per-pair loss on [P, TK] ----
            def seg(ap):
                return ap.rearrange("p (t k) -> p t k", k=K)

            # r_i = sqrt(1/(ss_i + eps))
            r_sb = cst.tile([P, T_TOT], f32)
            nc.vector.tensor_scalar_add(ssi_all[:], ssi_all[:], EPS)
            nc.vector.reciprocal(ssi_all[:], ssi_all[:])
            nc.scalar.activation(out=r_sb[:], in_=ssi_all[:], func=Act.Sqrt)

            # rn_j = sqrt(1/(ss_j + eps))
            rn = cst.tile([P, TK], f32)
            nc.vector.tensor_scalar_add(ssn_all[:], ssn_all[:], EPS)
            nc.vector.reciprocal(ssn_all[:], ssn_all[:])
            nc.scalar.activation(out=rn[:], in_=ssn_all[:], func=Act.Sqrt)

            # cos -> d2 -> dist (rn reused as scratch chain)
            nc.vector.tensor_tensor(out=rn[:], in0=dot_all[:], in1=rn[:],
                                    op=Alu.mult)
            nc.vector.tensor_tensor(out=seg(rn), in0=seg(rn),
                                    in1=r_sb[:, :, None].to_broadcast([P, T_TOT, K]),
                                    op=Alu.mult)
            nc.vector.tensor_scalar(rn[:], rn[:], -2.0, 2.0, Alu.mult, Alu.add)
            nc.vector.tensor_scalar_max(rn[:], rn[:], 0.0)
            eps_tile = cst.tile([P, 1], f32)
            nc.vector.memset(eps_tile[:], EPS)
            dist = dot_all  # dot no longer needed
            nc.scalar.activation(out=dist[:], in_=rn[:], func=Act.Sqrt,
                                 bias=eps_tile[:, 0:1])

            # M = -min_k dist; s = dist + M; e = exp(-s)
            M = cst.tile([P, T_TOT], f32)
            nc.vector.tensor_reduce(out=M[:], in_=seg(dist), axis=Ax.X,
                                    op=Alu.min, negate=True)
            s_t = dist
            nc.vector.tensor_tensor(out=seg(s_t), in0=seg(dist),
                                    in1=M[:, :, None].to_broadcast([P, T_TOT, K]),
                                    op=Alu.add)
            e_t = cst.tile([P, TK], f32)
            nc.scalar.activation(out=e_t[:], in_=s_t[:], func=Act.Exp, scale=-1.0)

            # posmask, npos, point_mask
            pos = tgn_sb  # overwrite labels with the mask
            nc.vector.tensor_tensor(out=seg(pos), in0=seg(tgn_sb),
                                    in1=tgts_sb[:, :, None].to_broadcast([P, T_TOT, K]),
                                    op=Alu.is_equal)
            npos = cst.tile([P, T_TOT], f32)
            nc.vector.tensor_reduce(out=npos[:], in_=seg(pos), axis=Ax.X, op=Alu.add)
            g1 = cst.tile([P, T_TOT], f32)
            pm = cst.tile([P, T_TOT], f32)
            nc.vector.tensor_scalar(g1[:], npos[:], 0.5, None, Alu.is_gt)
            nc.vector.tensor_scalar(pm[:], npos[:], K - 0.5, None, Alu.is_lt)
            nc.vector.tensor_tensor(out=pm[:], in0=g1[:], in1=pm[:], op=Alu.mult)

            # neg = sum(e) - sum(e*pos); under = e + neg; L = ln(under)
            sall = cst.tile([P, T_TOT], f32)
            nc.vector.tensor_reduce(out=sall[:], in_=seg(e_t), axis=Ax.X, op=Alu.add)
            ep = ssn_all  # scratch
            nc.vector.tensor_tensor(out=ep[:], in0=e_t[:], in1=pos[:], op=Alu.mult)
            spos = cst.tile([P, T_TOT], f32)
            nc.vector.tensor_reduce(out=spos[:], in_=seg(ep), axis=Ax.X, op=Alu.add)
            nc.vector.tensor_tensor(out=sall[:], in0=sall[:], in1=spos[:],
                                    op=Alu.subtract)
            nc.vector.tensor_tensor(out=seg(e_t), in0=seg(e_t),
                                    in1=sall[:, :, None].to_broadcast([P, T_TOT, K]),
                                    op=Alu.add)
            nc.scalar.activation(out=e_t[:], in_=e_t[:], func=Act.Ln)

            # per_pair = L + s ; contrib = per_pair * pos * pm
            nc.vector.tensor_tensor(out=e_t[:], in0=e_t[:], in1=s_t[:], op=Alu.add)
            nc.vector.tensor_tensor(out=seg(pos), in0=seg(pos),
                                    in1=pm[:, :, None].to_broadcast([P, T_TOT, K]),
                                    op=Alu.mult)
            nc.vector.tensor_tensor(out=e_t[:], in0=e_t[:], in1=pos[:], op=Alu.mult)

            # reduce to per-partition (sum, cnt), then across partitions via PE
            vals = cst.tile([P, 2], f32)
            nc.vector.tensor_reduce(out=vals[:, 0:1], in_=e_t[:], axis=Ax.X,
                                    op=Alu.add)
            nc.vector.tensor_reduce(out=vals[:, 1:2], in_=pos[:], axis=Ax.X,
                                    op=Alu.add)
            ones = cst.tile([P, 1], f32)
            nc.vector.memset(ones[:], 1.0)
            pst = psp.tile([2, 1], f32, space="PSUM")
            nc.tensor.matmul(out=pst[:], lhsT=vals[:], rhs=ones[:], start=True,
                             stop=True)
            res_sb = cst.tile([2, 1], f32)
            nc.vector.tensor_copy(out=res_sb[:], in_=pst[:])
            nc.sync.dma_start(out=part[:], in_=res_sb[:])
    nc.finalize()
    return nc


def make_in_maps(x, neighbor_idx, target, nsh=NSH, ncores=NCORES, fp16=False):
    """Shard + pad + per-chunk index compaction, host-side (data marshaling)."""
    import ml_dtypes

    bf = ml_dtypes.bfloat16
    x = np.ascontiguousarray(np.asarray(x, dtype=np.float32)).astype(bf)
    idx_all = np.asarray(neighbor_idx).astype(np.int64)
    tgtf = np.asarray(target).astype(np.float32)

    GGRP = max(CHUNK_TILES)
    USZ = GGRP * K * P

    in_maps = []
    for c in range(ncores):
        lo = c * nsh
        pts = np.arange(lo, lo + NPAD, dtype=np.int64)
        valid = pts < lo + nsh
        pts = np.where(valid, pts, lo)  # pad points alias point `lo`

        # point (p, t) of chunk g at xs row (t0 + t_l)*P ... laid out so the
        # chunk's block is contiguous: row = t0*P + p*grp + t_l
        xs_host = np.zeros((NPAD, C), dtype=bf)
        tgts_host = np.full((P, T_TOT), -1.0, dtype=np.float32)
        tgn_host = np.zeros((P, T_TOT, K), dtype=np.float32)
        tab_host = np.zeros((NCHUNK, USZ, C), dtype=bf)
        idx16_host = np.zeros((NCHUNK, P, USZ // 16), dtype=np.int16)

        for g, grp in enumerate(CHUNK_TILES):
            t0 = CHUNK_T0[g]
            nidx = grp * K * P
            # u in [0, grp*P): u -> (p = u//grp, t_l = u%grp)
            u = np.arange(grp * P)
            p_g, tl_g = u // grp, u % grp
            gpts = pts[t0 * P + u]
            xs_host[t0 * P + u] = x[gpts]
            t_glob = t0 + tl_g
            tgts_host[p_g, t_glob] = np.where(valid[t0 * P + u],
                                              tgtf[gpts], -1.0)
            refs = idx_all[gpts]                       # [grp*P, K]
            tgn_host[p_g, t_glob, :] = tgtf[refs]

            uniq = np.unique(refs)
            assert uniq.size <= USZ and uniq.size < 32768
            tab_host[g, :uniq.size] = x[uniq]
            lidx = np.searchsorted(uniq, refs).astype(np.int16)
            # gather position i = slot*128 + p, slot = t_l*K + k
            flat = np.zeros(nidx, dtype=np.int16)
            slot = tl_g[:, None] * K + np.arange(K)[None, :]
            flat[slot * P + p_g[:, None]] = lidx
            idx16_host[g, :, 0:nidx // 16] = np.tile(
                flat.reshape(nidx // 16, 16).T, (8, 1))

        in_maps.append({
            "xs": xs_host,
            "tab": tab_host,
            "idx16": idx16_host,
            "tgn": tgn_host.reshape(P, TK),
            "tgts": tgts_host,
        })
    return in_maps


def combine_parts(parts):
    parts = np.asarray(parts, dtype=np.float64)
    s = parts[:, 0].sum()
    cnt = parts[:, 1].sum()
    loss = s / max(cnt, 1.0) if cnt > 0 else 0.0
    return np.asarray(loss, dtype=np.float32)


def kernel(p, x, neighbor_idx, target):
    from concourse.bass_utils import run_bass_kernel_spmd

    in_maps = make_in_maps(x, neighbor_idx, target, fp16=FP16)
    nc = build_nc(fp16=FP16)
    res = run_bass_kernel_spmd(nc, in_maps, list(range(NCORES)))
    parts = [r["part"] for r in res.results]
    return combine_parts(parts)
